# revision 17
# baseline (speedup 1.0000x reference)
"""Trainium2 Bass kernel for nn_MultiHeadAttention (B=4, T=2048, D=1024,
H=16, d_k=64) on 8 NeuronCores.

Sharding: tensor-parallel over heads — core c computes heads {2c, 2c+1} for
ALL batches (W_q/W_k/W_v column-sharded, W_o row-sharded). The final
all-reduce of the output projection is replaced by a host-side sum of the 8
partial outputs. Per-batch attention length (ceil(valid_len/128) Tk tiles)
is baked into the single SPMD program, keeping every core's instruction
stream identical AND load-balanced (each core owns 2 heads of every batch).

v2 structure (vs the 353us baseline):
  - exp fused across the two heads: one [128, 1024] ACT instruction per
    (slot, tq-chunk, Tk-tile) reading a 2-bank PSUM tile; amortizes the
    ~350-cycle ACT fixed cost (was 2x [128,512]).
  - K/V projections (and their x-window DMAs) only cover ceil(J/4) 512-wide
    chunks per slot instead of all of T.
  - softmax denominator handled via the ones-column folded into V; den rows
    are DMA-gathered into a [*, 128] layout so reciprocal_approx_fast uses
    4 lanes/row; 1/den is broadcast across partitions with a K=1 matmul.
  - normalization + output projection pipelined per (slot, tq-chunk) right
    behind attention, so the kernel has no serial tail; PSUM->SBUF
    evacuations are split between ACT and DVE to balance the two engines.
  - trn2 encodes at most one semaphore wait per instruction; a post-pass
    splits any multi-wait instruction Tile emits into single-wait ops.
"""
import os
import sys

for _p in ("/opt/trn_rl_repo", "/root/.axon_site/_ro/trn_rl_repo"):
    if os.path.isdir(_p) and _p not in sys.path:
        sys.path.append(_p)

import numpy as np
import ml_dtypes

import concourse.bass as bass
import concourse.mybir as mybir
import concourse.tile as tile
from concourse.bass import ts
from concourse.bass_utils import run_bass_kernel_spmd

D = 1024
T = 2048
H = 16
DK = 64
P = 128
KC = D // P          # 8 contraction chunks for the projections
NT = T // 512        # 4 Tq chunks of 512
TC = T // P          # 16 Tk tiles / T chunks
NCORES = 8
CPB = (H // NCORES) * DK   # 128 projection cols per core (2 heads)
MASK_NEG = -30000.0

F32 = mybir.dt.float32
F32R = mybir.dt.float32r
BF16 = mybir.dt.bfloat16
AF = mybir.ActivationFunctionType
BF16_NP = ml_dtypes.bfloat16

# fraction of out-proj evacuations routed to ACT (rest on DVE); tuned from
# the engine-busy balance in the trace
OP_EVAC_ACT_EVERY = 6


def _split_multi_waits(nc):
    """trn2 instructions encode at most one sync wait; split the rest into
    standalone single-wait event-semaphore ops (see module docstring)."""
    n_split = 0
    for f in nc.m.functions:
        for blk in f.blocks:
            insts = blk.instructions
            out = []
            changed = False
            for inst in insts:
                si = inst.sync_info
                if si is not None and len(si.on_wait) > 1:
                    waits = list(si.on_wait)
                    for k, wt in enumerate(waits[:-1]):
                        ev = mybir.InstEventSemaphore(
                            name=f"{inst.name}_wsplit{k}",
                            engine=inst.engine,
                            ins=[],
                            outs=[],
                            bass_nofuse=True,
                            sync_info=mybir.SyncInfo(on_wait=[wt], on_update=[]),
                        )
                        out.append(ev)
                        n_split += 1
                    inst.sync_info = mybir.SyncInfo(
                        on_wait=[waits[-1]], on_update=si.on_update
                    )
                    changed = True
                out.append(inst)
            if changed:
                blk.instructions = out
    return n_split


def build_nc(NB, J_list, dt_x):
    """Build the SPMD program.

    NB     : number of batch slots handled per core
    J_list : per batch slot, number of 128-row Tk tiles of attention
    dt_x   : dtype of x/weights/intermediates (BF16)
    """
    NKC = [min(NT, -(-j // 4)) for j in J_list]  # K/V 512-chunks per slot
    nc = bass.Bass()

    # window-major layout: one [P, KC, 512] window is contiguous per
    # partition (8 KB runs) so each DMA needs only 128 descriptors
    xq_d = [nc.declare_dram_parameter(f"xq{s}", [NT, P, KC, 512], dt_x,
                                      isOutput=False) for s in range(NB)]
    xk_d = [nc.declare_dram_parameter(f"xk{s}", [NKC[s], P, KC, 512], dt_x,
                                      isOutput=False) for s in range(NB)]
    xv_d = [nc.declare_dram_parameter(f"xv{s}", [NKC[s], P, KC, 512], dt_x,
                                      isOutput=False) for s in range(NB)]
    wq_d = nc.declare_dram_parameter("wq", [P, KC, CPB], dt_x, isOutput=False)
    wk_d = nc.declare_dram_parameter("wk", [P, KC, CPB], dt_x, isOutput=False)
    wv_d = nc.declare_dram_parameter("wv", [P, KC, CPB], dt_x, isOutput=False)
    wo_d = nc.declare_dram_parameter("wo", [P, D], dt_x, isOutput=False)
    bq_d = nc.declare_dram_parameter("bq", [P, 1], F32, isOutput=False)
    bk_d = nc.declare_dram_parameter("bk", [P, 1], F32, isOutput=False)
    mb_d = [nc.declare_dram_parameter(f"mb{s}", [P, TC], F32, isOutput=False)
            for s in range(NB)]
    onesb_d = nc.declare_dram_parameter("onesb", [1, DK], F32R, isOutput=False)
    o_d = [nc.declare_dram_parameter(f"o{s}", [T, D], BF16, isOutput=True)
           for s in range(NB)]

    ev_ct = [0]

    with tile.TileContext(nc) as tc:
        with (
            tc.tile_pool(name="pers", bufs=1) as pers,
            tc.tile_pool(name="xw", bufs=6) as xw,
            tc.tile_pool(name="atp", bufs=4) as atp,
            tc.tile_pool(name="uop", bufs=4) as uop,
            tc.tile_pool(name="aop", bufs=3) as aop,
            tc.tile_pool(name="otp", bufs=2) as otp,
            tc.tile_pool(name="smp", bufs=4) as smp,
            tc.tile_pool(name="ps_qk", bufs=2, space="PSUM") as ps_qk,
            tc.tile_pool(name="ps_pv", bufs=2, space="PSUM") as ps_pv,
            tc.tile_pool(name="ps_ms", bufs=2, space="PSUM") as ps_ms,
        ):
            def evac(dst, src, on_act):
                if on_act:
                    nc.scalar.activation(dst, src, AF.Identity)
                else:
                    nc.vector.tensor_copy(dst, src)

            # ---- persistent tensors -------------------------------------
            wq = pers.tile([P, KC, CPB], dt_x, name="wq")
            wk = pers.tile([P, KC, CPB], dt_x, name="wk")
            wv = pers.tile([P, KC, CPB], dt_x, name="wv")
            wo = pers.tile([P, D], dt_x, name="wo")
            bq = pers.tile([P, 1], F32, name="bq")
            bk = pers.tile([P, 1], F32, name="bk")
            nc.sync.dma_start(wq[:], wq_d[:])
            nc.sync.dma_start(wk[:], wk_d[:])
            nc.sync.dma_start(wv[:], wv_d[:])
            nc.sync.dma_start(wo[:], wo_d[:])
            nc.sync.dma_start(bq[:], bq_d[:])
            nc.sync.dma_start(bk[:], bk_d[:])
            mb = []
            for s in range(NB):
                t = pers.tile([P, TC], F32, name=f"mb{s}")
                nc.sync.dma_start(t[:], mb_d[s][:])
                mb.append(t)

            ones_b = pers.tile([1, DK], F32R, name="ones_b")  # 1/den bcast lhsT
            nc.sync.dma_start(ones_b[:], onesb_d[:])

            QT = [pers.tile([P, T], dt_x, name=f"QT{s}") for s in range(NB)]
            KT = [pers.tile([P, NKC[s] * 512], dt_x, name=f"KT{s}")
                  for s in range(NB)]
            # V with a ones column folded in at free index DK of each head
            V = [pers.tile([P, 4 * NKC[s], 2, DK + 1], dt_x, name=f"V{s}")
                 for s in range(NB)]
            for s in range(NB):
                nc.vector.memset(V[s][:, :, :, DK], 1.0)



            def q_chunk(s, n):
                xq_w = xw.tile([P, KC, 512], dt_x, tag="xw", name="xq_w")
                nc.sync.dma_start(xq_w[:], xq_d[s][n])
                ps = ps_ms.tile([P, 512], F32, tag="ms", name="ps_q")
                for kc in range(KC):
                    nc.tensor.matmul(ps[:], wq[:, kc, :], xq_w[:, kc, :],
                                     start=(kc == 0), stop=(kc == KC - 1))
                nc.vector.tensor_scalar_add(QT[s][:, ts(n, 512)], ps[:],
                                            bq[:, 0:1])

            def k_chunk(s, c):
                xk_w = xw.tile([P, KC, 512], dt_x, tag="xw", name="xk_w")
                nc.sync.dma_start(xk_w[:], xk_d[s][c])
                ps = ps_ms.tile([P, 512], F32, tag="ms", name="ps_k")
                for kc in range(KC):
                    nc.tensor.matmul(ps[:], wk[:, kc, :], xk_w[:, kc, :],
                                     start=(kc == 0), stop=(kc == KC - 1))
                nc.vector.tensor_scalar_add(KT[s][:, ts(c, 512)], ps[:],
                                            bk[:, 0:1])

            def v_chunk(s, c):
                xv_w = xw.tile([P, KC, 512], dt_x, tag="xw", name="xv_w")
                nc.sync.dma_start(xv_w[:], xv_d[s][c])
                psv = ps_ms.tile([P, 512], F32, tag="ms", name="ps_v")
                for ml in range(4):
                    for kc in range(KC):
                        nc.tensor.matmul(psv[:, ts(ml, P)],
                                         xv_w[:, kc, ts(ml, P)],
                                         wv[:, kc, :],
                                         start=(kc == 0),
                                         stop=(kc == KC - 1))
                # [128, 4*128] -> V[s][:, 4c:4c+4, :, 0:DK]
                nc.vector.tensor_copy(
                    V[s][:, 4 * c:4 * c + 4, :, 0:DK],
                    psv[:].rearrange("p (m h d) -> p m h d", m=4, d=DK))

            def proj_chunks(s):
                for n in range(NT):
                    yield lambda n=n: q_chunk(s, n)
                for c in range(NKC[s]):
                    yield lambda c=c: k_chunk(s, c)
                    yield lambda c=c: v_chunk(s, c)

            def attn_tq(s, tq):
                """QK^T + exp + P@V for one (slot, 512-wide Tq chunk).
                Returns the two PV psum tiles (h0, h1)."""
                J = J_list[s]
                pvs = [ps_pv.tile([P, 512], F32, tag="pv", name=f"pv{h}")
                       for h in range(2)]
                for j in range(J):
                    ps = ps_qk.tile([P, 2, 512], F32, tag="qk")
                    nc.tensor.matmul(ps[:, 0, :],
                                     KT[s][0:DK, ts(j, P)],
                                     QT[s][0:DK, ts(tq, 512)],
                                     start=True, stop=True,
                                     tile_position=(0, 0))
                    nc.tensor.matmul(ps[:, 1, :],
                                     KT[s][DK:P, ts(j, P)],
                                     QT[s][DK:P, ts(tq, 512)],
                                     start=True, stop=True,
                                     tile_position=(DK, 0))
                    at = atp.tile([P, 2, 512], dt_x, tag="at")
                    nc.scalar.activation(at[:, :, :], ps[:, :, :], AF.Exp,
                                         bias=mb[s][:, j:j + 1], scale=0.125)
                    for h in range(2):
                        nc.tensor.matmul(pvs[h][0:DK + 1, :],
                                         V[s][:, j, h, :], at[:, h, :],
                                         start=(j == 0), stop=(j == J - 1))
                return pvs

            def norm_out(s, tq, pvs):
                """normalize + output-project one (slot, Tq chunk)."""
                uos = []
                # den rows gathered 16-partitions-wide so the (8 cyc/elem)
                # reciprocal runs 32 lanes x 32 elems instead of 2 x 512
                dn = smp.tile([32, 32], F32, tag="dn")
                for h in range(2):
                    uo = uop.tile([DK + 1, 512], F32, tag="uo")
                    evac(uo[:], pvs[h][0:DK + 1, :], on_act=(h == 0))
                    nc.sync.dma_start(dn[16 * h:16 * h + 16, :],
                                      uo[DK:DK + 1, :])
                    uos.append(uo)
                rc = smp.tile([32, 32], F32, tag="rc")
                with nc.allow_low_precision(
                        reason="f32r output is bit-identical to f32"):
                    nc.vector.reciprocal(rc[:], dn[:])
                ao = aop.tile([P, 512], dt_x, tag="ao")
                for h in range(2):
                    rst = smp.tile([1, 512], F32R, tag="rst")
                    nc.sync.dma_start(
                        rst[:], rc[16 * h:16 * h + 16, :].bitcast(F32R))
                    ps_b = ps_pv.tile([P, 512], F32, tag="pv", name="ps_b")
                    nc.tensor.matmul(ps_b[0:DK, :], ones_b[0:1, :],
                                     rst[0:1, :], start=True, stop=True)
                    nc.vector.tensor_mul(out=ao[ts(h, DK), :],
                                         in0=ps_b[0:DK, :],
                                         in1=uos[h][0:DK, :])
                ot = otp.tile([P, 4, D], BF16, tag="ot")
                for ml in range(4):
                    for n2 in range(2):
                        ps_o = ps_ms.tile([P, 512], F32, tag="ms")
                        nc.tensor.matmul(ps_o[:], ao[:, ts(ml, P)],
                                         wo[:, ts(n2, 512)],
                                         start=True, stop=True)
                        ev_ct[0] += 1
                        evac(ot[:, ml, ts(n2, 512)], ps_o[:],
                             on_act=(ev_ct[0] % OP_EVAC_ACT_EVERY == 0))
                nc.sync.dma_start(
                    o_d[s][ts(tq, 512), :].rearrange("(m p) d -> p m d", p=P),
                    ot[:])

            # ---- emission schedule --------------------------------------
            # slots 0,1 projected up front; later slots' projection chunks
            # drain 2 per (slot, tq) unit so PE always has independent
            # filler work while ACT paces the attention stream
            for fn in proj_chunks(0):
                fn()
            if NB > 1:
                for fn in proj_chunks(1):
                    fn()
            pend = {s: list(proj_chunks(s)) for s in range(2, NB)}

            def drain(n):
                while n > 0:
                    s2 = next((x for x in sorted(pend) if pend[x]), None)
                    if s2 is None:
                        return
                    pend[s2].pop(0)()
                    n -= 1

            for s in range(NB):
                for tq in range(NT):
                    pvs = attn_tq(s, tq)
                    drain(2)
                    if tq == NT - 1 and (s + 1) in pend:
                        # slot s+1 starts next: its chunks must all be out
                        drain(len(pend[s + 1]))
                        del pend[s + 1]
                    norm_out(s, tq, pvs)

    _split_multi_waits(nc)
    return nc


_CACHE = {}


def _get_nc(NB, J_list, dt_x):
    key = (NB, tuple(J_list), str(dt_x))
    if key not in _CACHE:
        _CACHE[key] = build_nc(NB, J_list, dt_x)
    return _CACHE[key]


def _xt(x, dt_np, nchunks=NT):
    """[T, D] -> [nchunks, P, KC, 512] transposed window-major layout."""
    xt = x.T.reshape(KC, P, NT, 512).transpose(2, 1, 0, 3)[:nchunks]
    return np.ascontiguousarray(xt).astype(dt_np)


def kernel(**inputs):
    query = np.asarray(inputs["query"], dtype=np.float32)
    key = np.asarray(inputs["key"], dtype=np.float32)
    value = np.asarray(inputs["value"], dtype=np.float32)
    vl = np.asarray(inputs["valid_length"]).astype(np.int64)
    W_q = np.asarray(inputs["W_q"], dtype=np.float32)
    b_q = np.asarray(inputs["b_q"], dtype=np.float32)
    W_k = np.asarray(inputs["W_k"], dtype=np.float32)
    b_k = np.asarray(inputs["b_k"], dtype=np.float32)
    W_v = np.asarray(inputs["W_v"], dtype=np.float32)
    b_v = np.asarray(inputs["b_v"], dtype=np.float32)
    W_o = np.asarray(inputs["W_o"], dtype=np.float32)
    b_o = np.asarray(inputs["b_o"], dtype=np.float32)

    B = query.shape[0]
    NB = B
    dt_x = BF16
    dt_np = BF16_NP

    # slot s handles batch order[s]; J (Tk tiles) baked per slot.
    # Ascending J: the longest-attention slot runs LAST so its (ACT-paced)
    # exp stream hides the normalization + output projection of every other
    # slot — the kernel ends with almost no PE/DVE-only tail.
    order = np.argsort(vl, kind="stable")
    J_list = []
    for s in range(NB):
        v = int(vl[order[s]])
        J_list.append(TC if v == 0 else max(1, -(-v // P)))
    NKC = [min(NT, -(-j // 4)) for j in J_list]

    nc = _get_nc(NB, J_list, dt_x)

    # host-side shard prep
    xq_np, xk_np, xv_np, mb_np = [], [], [], []
    for s in range(NB):
        b = int(order[s])
        v = int(vl[b])
        q_b = query[b] if v != 0 else np.zeros_like(query[b])
        xq_np.append(_xt(q_b, dt_np))
        xk_np.append(_xt(key[b], dt_np, NKC[s]))
        xv_np.append(_xt(value[b], dt_np, NKC[s]))
        rows = np.arange(P)[:, None] + P * np.arange(TC)[None, :]
        if v == 0:
            m = np.zeros((P, TC), np.float32)
        else:
            m = np.where(rows < v, 0.0, MASK_NEG).astype(np.float32)
        mb_np.append(m)

    in_maps = []
    for c in range(NCORES):
        c0 = c * CPB
        cols = slice(c0, c0 + CPB)
        im = {
            "wq": np.ascontiguousarray(
                W_q.reshape(KC, P, H * DK).transpose(1, 0, 2)[:, :, cols]
            ).astype(dt_np),
            "wk": np.ascontiguousarray(
                W_k.reshape(KC, P, H * DK).transpose(1, 0, 2)[:, :, cols]
            ).astype(dt_np),
            "wv": np.ascontiguousarray(
                W_v.reshape(KC, P, H * DK).transpose(1, 0, 2)[:, :, cols]
            ).astype(dt_np),
            "wo": np.ascontiguousarray(W_o[cols]).astype(dt_np),
            "bq": np.ascontiguousarray(b_q[cols][:, None]).astype(np.float32),
            "bk": np.ascontiguousarray(b_k[cols][:, None]).astype(np.float32),
            "onesb": np.ones((1, DK), np.float32),
        }
        for s in range(NB):
            im[f"xq{s}"] = xq_np[s]
            im[f"xk{s}"] = xk_np[s]
            im[f"xv{s}"] = xv_np[s]
            im[f"mb{s}"] = mb_np[s]
        in_maps.append(im)

    res = run_bass_kernel_spmd(nc, in_maps, list(range(NCORES)))

    # b_v is not applied on device; softmax weights sum to 1 so it adds
    # exactly b_v @ W_o to every output row
    bo_eff = b_o + b_v @ W_o
    out = np.zeros((B, T, D), np.float32)
    for s in range(NB):
        b = int(order[s])
        acc = np.zeros((T, D), np.float32)
        for c in range(NCORES):
            acc += np.asarray(res.results[c][f"o{s}"]).astype(np.float32)
        out[b] = acc + bo_eff[None, :]
    return out


# revision 18
# speedup vs baseline: 1.1649x; 1.1649x over previous
"""Trainium2 Bass kernel for nn_MultiHeadAttention (B=4, T=2048, D=1024,
H=16, d_k=64) on 8 NeuronCores.

Sharding: tensor-parallel over heads — core c computes heads {2c, 2c+1} for
ALL batches (W_q/W_k/W_v column-sharded, W_o row-sharded). The final
all-reduce of the output projection is replaced by a host-side sum of the 8
partial outputs. Per-batch attention length (ceil(valid_len/128) Tk tiles)
is baked into the single SPMD program, keeping every core's instruction
stream identical AND load-balanced (each core owns 2 heads of every batch).

v2 structure (vs the 353us baseline):
  - exp fused across the two heads: one [128, 1024] ACT instruction per
    (slot, tq-chunk, Tk-tile) reading a 2-bank PSUM tile; amortizes the
    ~350-cycle ACT fixed cost (was 2x [128,512]).
  - K/V projections (and their x-window DMAs) only cover ceil(J/4) 512-wide
    chunks per slot instead of all of T.
  - softmax denominator handled via the ones-column folded into V; den rows
    are DMA-gathered into a [*, 128] layout so reciprocal_approx_fast uses
    4 lanes/row; 1/den is broadcast across partitions with a K=1 matmul.
  - normalization + output projection pipelined per (slot, tq-chunk) right
    behind attention, so the kernel has no serial tail; PSUM->SBUF
    evacuations are split between ACT and DVE to balance the two engines.
  - trn2 encodes at most one semaphore wait per instruction; a post-pass
    splits any multi-wait instruction Tile emits into single-wait ops.
"""
import os
import sys

for _p in ("/opt/trn_rl_repo", "/root/.axon_site/_ro/trn_rl_repo"):
    if os.path.isdir(_p) and _p not in sys.path:
        sys.path.append(_p)

import numpy as np
import ml_dtypes

import concourse.bass as bass
import concourse.mybir as mybir
import concourse.tile as tile
from concourse.bass import ts
from concourse.bass_utils import run_bass_kernel_spmd

D = 1024
T = 2048
H = 16
DK = 64
P = 128
KC = D // P          # 8 contraction chunks for the projections
NT = T // 512        # 4 Tq chunks of 512
TC = T // P          # 16 Tk tiles / T chunks
NCORES = 8
CPB = (H // NCORES) * DK   # 128 projection cols per core (2 heads)
MASK_NEG = -30000.0

F32 = mybir.dt.float32
F32R = mybir.dt.float32r
BF16 = mybir.dt.bfloat16
AF = mybir.ActivationFunctionType
BF16_NP = ml_dtypes.bfloat16

# fraction of out-proj evacuations routed to ACT (rest on DVE); tuned from
# the engine-busy balance in the trace
OP_EVAC_ACT_EVERY = 6


def _split_multi_waits(nc):
    """trn2 instructions encode at most one sync wait; split the rest into
    standalone single-wait event-semaphore ops (see module docstring)."""
    n_split = 0
    for f in nc.m.functions:
        for blk in f.blocks:
            insts = blk.instructions
            out = []
            changed = False
            for inst in insts:
                si = inst.sync_info
                if si is not None and len(si.on_wait) > 1:
                    waits = list(si.on_wait)
                    for k, wt in enumerate(waits[:-1]):
                        ev = mybir.InstEventSemaphore(
                            name=f"{inst.name}_wsplit{k}",
                            engine=inst.engine,
                            ins=[],
                            outs=[],
                            bass_nofuse=True,
                            sync_info=mybir.SyncInfo(on_wait=[wt], on_update=[]),
                        )
                        out.append(ev)
                        n_split += 1
                    inst.sync_info = mybir.SyncInfo(
                        on_wait=[waits[-1]], on_update=si.on_update
                    )
                    changed = True
                out.append(inst)
            if changed:
                blk.instructions = out
    return n_split


def build_nc(NB, J_list, dt_x):
    """Build the SPMD program.

    NB     : number of batch slots handled per core
    J_list : per batch slot, number of 128-row Tk tiles of attention
    dt_x   : dtype of x/weights/intermediates (BF16)
    """
    NKC = [min(NT, -(-j // 4)) for j in J_list]  # K/V 512-chunks per slot
    nc = bass.Bass()

    # window-major layout: one [P, KC, 512] window is contiguous per
    # partition (8 KB runs) so each DMA needs only 128 descriptors
    xq_d = [nc.declare_dram_parameter(f"xq{s}", [NT, P, KC, 512], dt_x,
                                      isOutput=False) for s in range(NB)]
    xk_d = [nc.declare_dram_parameter(f"xk{s}", [NKC[s], P, KC, 512], dt_x,
                                      isOutput=False) for s in range(NB)]
    xv_d = [nc.declare_dram_parameter(f"xv{s}", [NKC[s], P, KC, 512], dt_x,
                                      isOutput=False) for s in range(NB)]
    wq_d = nc.declare_dram_parameter("wq", [P, KC, CPB], dt_x, isOutput=False)
    wk_d = nc.declare_dram_parameter("wk", [P, KC, CPB], dt_x, isOutput=False)
    wv_d = nc.declare_dram_parameter("wv", [P, KC, CPB], dt_x, isOutput=False)
    wo_d = nc.declare_dram_parameter("wo", [P, D], dt_x, isOutput=False)
    bq_d = nc.declare_dram_parameter("bq", [P, 1], F32, isOutput=False)
    bk_d = nc.declare_dram_parameter("bk", [P, 1], F32, isOutput=False)
    mb_d = [nc.declare_dram_parameter(f"mb{s}", [P, TC], F32, isOutput=False)
            for s in range(NB)]
    onesb_d = nc.declare_dram_parameter("onesb", [1, DK], F32R, isOutput=False)
    o_d = [nc.declare_dram_parameter(f"o{s}", [T, D], BF16, isOutput=True)
           for s in range(NB)]

    ev_ct = [0]

    with tile.TileContext(nc) as tc:
        with (
            tc.tile_pool(name="pers", bufs=1) as pers,
            tc.tile_pool(name="xw", bufs=6) as xw,
            tc.tile_pool(name="atp", bufs=4) as atp,
            tc.tile_pool(name="uop", bufs=4) as uop,
            tc.tile_pool(name="aop", bufs=3) as aop,
            tc.tile_pool(name="otp", bufs=2) as otp,
            tc.tile_pool(name="smp", bufs=4) as smp,
            tc.tile_pool(name="ps_qk", bufs=2, space="PSUM") as ps_qk,
            tc.tile_pool(name="ps_pv", bufs=2, space="PSUM") as ps_pv,
            tc.tile_pool(name="ps_ms", bufs=2, space="PSUM") as ps_ms,
        ):
            def evac(dst, src, on_act):
                if on_act:
                    nc.scalar.activation(dst, src, AF.Identity)
                else:
                    nc.vector.tensor_copy(dst, src)

            # ---- persistent tensors -------------------------------------
            wq = pers.tile([P, KC, CPB], dt_x, name="wq")
            wk = pers.tile([P, KC, CPB], dt_x, name="wk")
            wv = pers.tile([P, KC, CPB], dt_x, name="wv")
            wo = pers.tile([P, D], dt_x, name="wo")
            bq = pers.tile([P, 1], F32, name="bq")
            bk = pers.tile([P, 1], F32, name="bk")
            nc.sync.dma_start(wq[:], wq_d[:])
            nc.sync.dma_start(wk[:], wk_d[:])
            nc.sync.dma_start(wv[:], wv_d[:])
            nc.sync.dma_start(wo[:], wo_d[:])
            nc.sync.dma_start(bq[:], bq_d[:])
            nc.sync.dma_start(bk[:], bk_d[:])
            mb = []
            for s in range(NB):
                t = pers.tile([P, TC], F32, name=f"mb{s}")
                nc.sync.dma_start(t[:], mb_d[s][:])
                mb.append(t)

            ones_b = pers.tile([1, DK], F32R, name="ones_b")  # 1/den bcast lhsT
            nc.sync.dma_start(ones_b[:], onesb_d[:])

            QT = [pers.tile([P, T], dt_x, name=f"QT{s}") for s in range(NB)]
            KT = [pers.tile([P, NKC[s] * 512], dt_x, name=f"KT{s}")
                  for s in range(NB)]
            # V with a ones column folded in at free index DK of each head
            V = [pers.tile([P, 4 * NKC[s], 2, DK + 1], dt_x, name=f"V{s}")
                 for s in range(NB)]
            for s in range(NB):
                nc.vector.memset(V[s][:, :, :, DK], 1.0)



            def q_chunk(s, n):
                xq_w = xw.tile([P, KC, 512], dt_x, tag="xw", name="xq_w")
                nc.sync.dma_start(xq_w[:], xq_d[s][n])
                ps = ps_ms.tile([P, 512], F32, tag="ms", name="ps_q")
                for kc in range(KC):
                    nc.tensor.matmul(ps[:], wq[:, kc, :], xq_w[:, kc, :],
                                     start=(kc == 0), stop=(kc == KC - 1))
                nc.vector.tensor_scalar_add(QT[s][:, ts(n, 512)], ps[:],
                                            bq[:, 0:1])

            def k_chunk(s, c):
                xk_w = xw.tile([P, KC, 512], dt_x, tag="xw", name="xk_w")
                nc.sync.dma_start(xk_w[:], xk_d[s][c])
                ps = ps_ms.tile([P, 512], F32, tag="ms", name="ps_k")
                for kc in range(KC):
                    nc.tensor.matmul(ps[:], wk[:, kc, :], xk_w[:, kc, :],
                                     start=(kc == 0), stop=(kc == KC - 1))
                nc.vector.tensor_scalar_add(KT[s][:, ts(c, 512)], ps[:],
                                            bk[:, 0:1])

            def v_chunk(s, c):
                xv_w = xw.tile([P, KC, 512], dt_x, tag="xw", name="xv_w")
                nc.sync.dma_start(xv_w[:], xv_d[s][c])
                psv = ps_ms.tile([P, 512], F32, tag="ms", name="ps_v")
                for ml in range(4):
                    for kc in range(KC):
                        nc.tensor.matmul(psv[:, ts(ml, P)],
                                         xv_w[:, kc, ts(ml, P)],
                                         wv[:, kc, :],
                                         start=(kc == 0),
                                         stop=(kc == KC - 1))
                # [128, 4*128] -> V[s][:, 4c:4c+4, :, 0:DK]
                nc.vector.tensor_copy(
                    V[s][:, 4 * c:4 * c + 4, :, 0:DK],
                    psv[:].rearrange("p (m h d) -> p m h d", m=4, d=DK))

            def proj_chunks(s):
                for n in range(NT):
                    yield lambda n=n: q_chunk(s, n)
                for c in range(NKC[s]):
                    yield lambda c=c: k_chunk(s, c)
                    yield lambda c=c: v_chunk(s, c)

            def attn_tq(s, tq):
                """QK^T + exp + P@V for one (slot, 512-wide Tq chunk),
                plus PV-psum evacuation (frees the pv bufs for the next
                unit) and the den-row gather. Returns (uo_h0, uo_h1, dn)."""
                J = J_list[s]
                pvs = [ps_pv.tile([P, 512], F32, tag="pv", name=f"pv{h}")
                       for h in range(2)]
                for j in range(J):
                    ps = ps_qk.tile([P, 2, 512], F32, tag="qk")
                    nc.tensor.matmul(ps[:, 0, :],
                                     KT[s][0:DK, ts(j, P)],
                                     QT[s][0:DK, ts(tq, 512)],
                                     start=True, stop=True,
                                     tile_position=(0, 0))
                    nc.tensor.matmul(ps[:, 1, :],
                                     KT[s][DK:P, ts(j, P)],
                                     QT[s][DK:P, ts(tq, 512)],
                                     start=True, stop=True,
                                     tile_position=(DK, 0))
                    at = atp.tile([P, 2, 512], dt_x, tag="at")
                    nc.scalar.activation(at[:, :, :], ps[:, :, :], AF.Exp,
                                         bias=mb[s][:, j:j + 1], scale=0.125)
                    for h in range(2):
                        nc.tensor.matmul(pvs[h][0:DK + 1, :],
                                         V[s][:, j, h, :], at[:, h, :],
                                         start=(j == 0), stop=(j == J - 1))
                uos = []
                dn = smp.tile([32, 32], F32, tag="dn")
                for h in range(2):
                    uo = uop.tile([DK + 1, 512], F32, tag="uo")
                    evac(uo[:], pvs[h][0:DK + 1, :], on_act=(h == 0))
                    nc.sync.dma_start(dn[16 * h:16 * h + 16, :],
                                      uo[DK:DK + 1, :])
                    uos.append(uo)
                return uos, dn

            def norm_out(s, tq, unit):
                """normalize + output-project one (slot, Tq chunk)."""
                uos, dn = unit
                rc = smp.tile([32, 32], F32, tag="rc")
                with nc.allow_low_precision(
                        reason="f32r output is bit-identical to f32"):
                    nc.vector.reciprocal(rc[:], dn[:])
                ao = aop.tile([P, 512], dt_x, tag="ao")
                for h in range(2):
                    rst = smp.tile([1, 512], F32R, tag="rst")
                    nc.sync.dma_start(
                        rst[:], rc[16 * h:16 * h + 16, :].bitcast(F32R))
                    ps_b = ps_pv.tile([P, 512], F32, tag="pv", name="ps_b")
                    nc.tensor.matmul(ps_b[0:DK, :], ones_b[0:1, :],
                                     rst[0:1, :], start=True, stop=True)
                    nc.vector.tensor_mul(out=ao[ts(h, DK), :],
                                         in0=ps_b[0:DK, :],
                                         in1=uos[h][0:DK, :])
                ot = otp.tile([P, 4, D], BF16, tag="ot")
                for ml in range(4):
                    for n2 in range(2):
                        ps_o = ps_ms.tile([P, 512], F32, tag="ms")
                        nc.tensor.matmul(ps_o[:], ao[:, ts(ml, P)],
                                         wo[:, ts(n2, 512)],
                                         start=True, stop=True)
                        ev_ct[0] += 1
                        evac(ot[:, ml, ts(n2, 512)], ps_o[:],
                             on_act=(ev_ct[0] % OP_EVAC_ACT_EVERY == 0))
                nc.sync.dma_start(
                    o_d[s][ts(tq, 512), :].rearrange("(m p) d -> p m d", p=P),
                    ot[:])

            # ---- emission schedule --------------------------------------
            # slots 0,1 projected up front; later slots' projection chunks
            # drain 2 per (slot, tq) unit so PE always has independent
            # filler work while ACT paces the attention stream
            for fn in proj_chunks(0):
                fn()
            if NB > 1:
                for fn in proj_chunks(1):
                    fn()
            pend = {s: list(proj_chunks(s)) for s in range(2, NB)}

            def drain(n):
                while n > 0:
                    s2 = next((x for x in sorted(pend) if pend[x]), None)
                    if s2 is None:
                        return
                    pend[s2].pop(0)()
                    n -= 1

            # one-unit software pipeline: unit u's normalization +
            # output projection is emitted AFTER unit u+1's attention, so
            # its serial latency chain (DMA->recip->DMA->bcast->mul->MMs)
            # hides under the next unit's exp stream instead of stalling
            # every engine at each unit boundary
            prev = None
            for s in range(NB):
                for tq in range(NT):
                    unit = attn_tq(s, tq)
                    drain(2)
                    if tq == NT - 1 and (s + 1) in pend:
                        # slot s+1 starts next: its chunks must all be out
                        drain(len(pend[s + 1]))
                        del pend[s + 1]
                    if prev is not None:
                        norm_out(*prev)
                    prev = (s, tq, unit)
            norm_out(*prev)

    _split_multi_waits(nc)
    return nc


_CACHE = {}


def _get_nc(NB, J_list, dt_x):
    key = (NB, tuple(J_list), str(dt_x))
    if key not in _CACHE:
        _CACHE[key] = build_nc(NB, J_list, dt_x)
    return _CACHE[key]


def _xt(x, dt_np, nchunks=NT):
    """[T, D] -> [nchunks, P, KC, 512] transposed window-major layout."""
    xt = x.T.reshape(KC, P, NT, 512).transpose(2, 1, 0, 3)[:nchunks]
    return np.ascontiguousarray(xt).astype(dt_np)


def kernel(**inputs):
    query = np.asarray(inputs["query"], dtype=np.float32)
    key = np.asarray(inputs["key"], dtype=np.float32)
    value = np.asarray(inputs["value"], dtype=np.float32)
    vl = np.asarray(inputs["valid_length"]).astype(np.int64)
    W_q = np.asarray(inputs["W_q"], dtype=np.float32)
    b_q = np.asarray(inputs["b_q"], dtype=np.float32)
    W_k = np.asarray(inputs["W_k"], dtype=np.float32)
    b_k = np.asarray(inputs["b_k"], dtype=np.float32)
    W_v = np.asarray(inputs["W_v"], dtype=np.float32)
    b_v = np.asarray(inputs["b_v"], dtype=np.float32)
    W_o = np.asarray(inputs["W_o"], dtype=np.float32)
    b_o = np.asarray(inputs["b_o"], dtype=np.float32)

    B = query.shape[0]
    NB = B
    dt_x = BF16
    dt_np = BF16_NP

    # slot s handles batch order[s]; J (Tk tiles) baked per slot.
    # Ascending J: the longest-attention slot runs LAST so its (ACT-paced)
    # exp stream hides the normalization + output projection of every other
    # slot — the kernel ends with almost no PE/DVE-only tail.
    order = np.argsort(vl, kind="stable")
    J_list = []
    for s in range(NB):
        v = int(vl[order[s]])
        J_list.append(TC if v == 0 else max(1, -(-v // P)))
    NKC = [min(NT, -(-j // 4)) for j in J_list]

    nc = _get_nc(NB, J_list, dt_x)

    # host-side shard prep
    xq_np, xk_np, xv_np, mb_np = [], [], [], []
    for s in range(NB):
        b = int(order[s])
        v = int(vl[b])
        q_b = query[b] if v != 0 else np.zeros_like(query[b])
        xq_np.append(_xt(q_b, dt_np))
        xk_np.append(_xt(key[b], dt_np, NKC[s]))
        xv_np.append(_xt(value[b], dt_np, NKC[s]))
        rows = np.arange(P)[:, None] + P * np.arange(TC)[None, :]
        if v == 0:
            m = np.zeros((P, TC), np.float32)
        else:
            m = np.where(rows < v, 0.0, MASK_NEG).astype(np.float32)
        mb_np.append(m)

    in_maps = []
    for c in range(NCORES):
        c0 = c * CPB
        cols = slice(c0, c0 + CPB)
        im = {
            "wq": np.ascontiguousarray(
                W_q.reshape(KC, P, H * DK).transpose(1, 0, 2)[:, :, cols]
            ).astype(dt_np),
            "wk": np.ascontiguousarray(
                W_k.reshape(KC, P, H * DK).transpose(1, 0, 2)[:, :, cols]
            ).astype(dt_np),
            "wv": np.ascontiguousarray(
                W_v.reshape(KC, P, H * DK).transpose(1, 0, 2)[:, :, cols]
            ).astype(dt_np),
            "wo": np.ascontiguousarray(W_o[cols]).astype(dt_np),
            "bq": np.ascontiguousarray(b_q[cols][:, None]).astype(np.float32),
            "bk": np.ascontiguousarray(b_k[cols][:, None]).astype(np.float32),
            "onesb": np.ones((1, DK), np.float32),
        }
        for s in range(NB):
            im[f"xq{s}"] = xq_np[s]
            im[f"xk{s}"] = xk_np[s]
            im[f"xv{s}"] = xv_np[s]
            im[f"mb{s}"] = mb_np[s]
        in_maps.append(im)

    res = run_bass_kernel_spmd(nc, in_maps, list(range(NCORES)))

    # b_v is not applied on device; softmax weights sum to 1 so it adds
    # exactly b_v @ W_o to every output row
    bo_eff = b_o + b_v @ W_o
    out = np.zeros((B, T, D), np.float32)
    for s in range(NB):
        b = int(order[s])
        acc = np.zeros((T, D), np.float32)
        for c in range(NCORES):
            acc += np.asarray(res.results[c][f"o{s}"]).astype(np.float32)
        out[b] = acc + bo_eff[None, :]
    return out


# revision 19
# speedup vs baseline: 1.2064x; 1.0357x over previous
"""Trainium2 Bass kernel for nn_MultiHeadAttention (B=4, T=2048, D=1024,
H=16, d_k=64) on 8 NeuronCores.

Sharding: tensor-parallel over heads — core c computes heads {2c, 2c+1} for
ALL batches (W_q/W_k/W_v column-sharded, W_o row-sharded). The final
all-reduce of the output projection is replaced by a host-side sum of the 8
partial outputs. Per-batch attention length (ceil(valid_len/128) Tk tiles)
is baked into the single SPMD program, keeping every core's instruction
stream identical AND load-balanced (each core owns 2 heads of every batch).

v2 structure (vs the 353us baseline):
  - exp fused across the two heads: one [128, 1024] ACT instruction per
    (slot, tq-chunk, Tk-tile) reading a 2-bank PSUM tile; amortizes the
    ~350-cycle ACT fixed cost (was 2x [128,512]).
  - K/V projections (and their x-window DMAs) only cover ceil(J/4) 512-wide
    chunks per slot instead of all of T.
  - softmax denominator handled via the ones-column folded into V; den rows
    are DMA-gathered into a [*, 128] layout so reciprocal_approx_fast uses
    4 lanes/row; 1/den is broadcast across partitions with a K=1 matmul.
  - normalization + output projection pipelined per (slot, tq-chunk) right
    behind attention, so the kernel has no serial tail; PSUM->SBUF
    evacuations are split between ACT and DVE to balance the two engines.
  - trn2 encodes at most one semaphore wait per instruction; a post-pass
    splits any multi-wait instruction Tile emits into single-wait ops.
"""
import os
import sys

for _p in ("/opt/trn_rl_repo", "/root/.axon_site/_ro/trn_rl_repo"):
    if os.path.isdir(_p) and _p not in sys.path:
        sys.path.append(_p)

import numpy as np
import ml_dtypes

import concourse.bass as bass
import concourse.mybir as mybir
import concourse.tile as tile
from concourse.bass import ts
from concourse.bass_utils import run_bass_kernel_spmd

D = 1024
T = 2048
H = 16
DK = 64
P = 128
KC = D // P          # 8 contraction chunks for the projections
NT = T // 512        # 4 Tq chunks of 512
TC = T // P          # 16 Tk tiles / T chunks
NCORES = 8
CPB = (H // NCORES) * DK   # 128 projection cols per core (2 heads)
MASK_NEG = -30000.0

F32 = mybir.dt.float32
F32R = mybir.dt.float32r
BF16 = mybir.dt.bfloat16
AF = mybir.ActivationFunctionType
BF16_NP = ml_dtypes.bfloat16

# fraction of out-proj evacuations routed to ACT (rest on DVE); tuned from
# the engine-busy balance in the trace
OP_EVAC_ACT_EVERY = 6


def _split_multi_waits(nc):
    """trn2 instructions encode at most one sync wait; split the rest into
    standalone single-wait event-semaphore ops (see module docstring)."""
    n_split = 0
    for f in nc.m.functions:
        for blk in f.blocks:
            insts = blk.instructions
            out = []
            changed = False
            for inst in insts:
                si = inst.sync_info
                if si is not None and len(si.on_wait) > 1:
                    waits = list(si.on_wait)
                    for k, wt in enumerate(waits[:-1]):
                        ev = mybir.InstEventSemaphore(
                            name=f"{inst.name}_wsplit{k}",
                            engine=inst.engine,
                            ins=[],
                            outs=[],
                            bass_nofuse=True,
                            sync_info=mybir.SyncInfo(on_wait=[wt], on_update=[]),
                        )
                        out.append(ev)
                        n_split += 1
                    inst.sync_info = mybir.SyncInfo(
                        on_wait=[waits[-1]], on_update=si.on_update
                    )
                    changed = True
                out.append(inst)
            if changed:
                blk.instructions = out
    return n_split


def build_nc(NB, J_list, dt_x):
    """Build the SPMD program.

    NB     : number of batch slots handled per core
    J_list : per batch slot, number of 128-row Tk tiles of attention
    dt_x   : dtype of x/weights/intermediates (BF16)
    """
    NKC = [min(NT, -(-j // 4)) for j in J_list]  # K/V 512-chunks per slot
    nc = bass.Bass()

    # window-major layout: one [P, KC, 512] window is contiguous per
    # partition (8 KB runs) so each DMA needs only 128 descriptors
    xq_d = [nc.declare_dram_parameter(f"xq{s}", [NT, P, KC, 512], dt_x,
                                      isOutput=False) for s in range(NB)]
    xk_d = [nc.declare_dram_parameter(f"xk{s}", [NKC[s], P, KC, 512], dt_x,
                                      isOutput=False) for s in range(NB)]
    xv_d = [nc.declare_dram_parameter(f"xv{s}", [NKC[s], P, KC, 512], dt_x,
                                      isOutput=False) for s in range(NB)]
    wq_d = nc.declare_dram_parameter("wq", [P, KC, CPB], dt_x, isOutput=False)
    wk_d = nc.declare_dram_parameter("wk", [P, KC, CPB], dt_x, isOutput=False)
    wv_d = nc.declare_dram_parameter("wv", [P, KC, CPB], dt_x, isOutput=False)
    wo_d = nc.declare_dram_parameter("wo", [P, D], dt_x, isOutput=False)
    bq_d = nc.declare_dram_parameter("bq", [P, 1], F32, isOutput=False)
    bk_d = nc.declare_dram_parameter("bk", [P, 1], F32, isOutput=False)
    mb_d = [nc.declare_dram_parameter(f"mb{s}", [P, TC], F32, isOutput=False)
            for s in range(NB)]
    onesb_d = nc.declare_dram_parameter("onesb", [1, DK], F32R, isOutput=False)
    o_d = [nc.declare_dram_parameter(f"o{s}", [T, D], BF16, isOutput=True)
           for s in range(NB)]

    ev_ct = [0]

    with tile.TileContext(nc) as tc:
        with (
            tc.tile_pool(name="pers", bufs=1) as pers,
            tc.tile_pool(name="xw", bufs=6) as xw,
            tc.tile_pool(name="atp", bufs=4) as atp,
            tc.tile_pool(name="uop", bufs=6) as uop,
            tc.tile_pool(name="aop", bufs=4) as aop,
            tc.tile_pool(name="otp", bufs=3) as otp,
            tc.tile_pool(name="smp", bufs=6) as smp,
            tc.tile_pool(name="ps_qk", bufs=2, space="PSUM") as ps_qk,
            tc.tile_pool(name="ps_pv", bufs=2, space="PSUM") as ps_pv,
            tc.tile_pool(name="ps_ms", bufs=2, space="PSUM") as ps_ms,
        ):
            def evac(dst, src, on_act):
                if on_act:
                    nc.scalar.activation(dst, src, AF.Identity)
                else:
                    nc.vector.tensor_copy(dst, src)

            # ---- persistent tensors -------------------------------------
            wq = pers.tile([P, KC, CPB], dt_x, name="wq")
            wk = pers.tile([P, KC, CPB], dt_x, name="wk")
            wv = pers.tile([P, KC, CPB], dt_x, name="wv")
            wo = pers.tile([P, D], dt_x, name="wo")
            bq = pers.tile([P, 1], F32, name="bq")
            bk = pers.tile([P, 1], F32, name="bk")
            nc.sync.dma_start(wq[:], wq_d[:])
            nc.sync.dma_start(wk[:], wk_d[:])
            nc.sync.dma_start(wv[:], wv_d[:])
            nc.sync.dma_start(wo[:], wo_d[:])
            nc.sync.dma_start(bq[:], bq_d[:])
            nc.sync.dma_start(bk[:], bk_d[:])
            mb = []
            for s in range(NB):
                t = pers.tile([P, TC], F32, name=f"mb{s}")
                nc.sync.dma_start(t[:], mb_d[s][:])
                mb.append(t)

            ones_b = pers.tile([1, DK], F32R, name="ones_b")  # 1/den bcast lhsT
            nc.sync.dma_start(ones_b[:], onesb_d[:])

            QT = [pers.tile([P, T], dt_x, name=f"QT{s}") for s in range(NB)]
            KT = [pers.tile([P, NKC[s] * 512], dt_x, name=f"KT{s}")
                  for s in range(NB)]
            # V with a ones column folded in at free index DK of each head
            V = [pers.tile([P, 4 * NKC[s], 2, DK + 1], dt_x, name=f"V{s}")
                 for s in range(NB)]
            for s in range(NB):
                nc.vector.memset(V[s][:, :, :, DK], 1.0)



            def q_chunk(s, n):
                xq_w = xw.tile([P, KC, 512], dt_x, tag="xw", name="xq_w")
                nc.sync.dma_start(xq_w[:], xq_d[s][n])
                ps = ps_ms.tile([P, 512], F32, tag="ms", name="ps_q")
                for kc in range(KC):
                    nc.tensor.matmul(ps[:], wq[:, kc, :], xq_w[:, kc, :],
                                     start=(kc == 0), stop=(kc == KC - 1))
                nc.vector.tensor_scalar_add(QT[s][:, ts(n, 512)], ps[:],
                                            bq[:, 0:1])

            def k_chunk(s, c):
                xk_w = xw.tile([P, KC, 512], dt_x, tag="xw", name="xk_w")
                nc.sync.dma_start(xk_w[:], xk_d[s][c])
                ps = ps_ms.tile([P, 512], F32, tag="ms", name="ps_k")
                for kc in range(KC):
                    nc.tensor.matmul(ps[:], wk[:, kc, :], xk_w[:, kc, :],
                                     start=(kc == 0), stop=(kc == KC - 1))
                nc.vector.tensor_scalar_add(KT[s][:, ts(c, 512)], ps[:],
                                            bk[:, 0:1])

            def v_chunk(s, c):
                xv_w = xw.tile([P, KC, 512], dt_x, tag="xw", name="xv_w")
                nc.sync.dma_start(xv_w[:], xv_d[s][c])
                psv = ps_ms.tile([P, 512], F32, tag="ms", name="ps_v")
                for ml in range(4):
                    for kc in range(KC):
                        nc.tensor.matmul(psv[:, ts(ml, P)],
                                         xv_w[:, kc, ts(ml, P)],
                                         wv[:, kc, :],
                                         start=(kc == 0),
                                         stop=(kc == KC - 1))
                # [128, 4*128] -> V[s][:, 4c:4c+4, :, 0:DK]
                nc.vector.tensor_copy(
                    V[s][:, 4 * c:4 * c + 4, :, 0:DK],
                    psv[:].rearrange("p (m h d) -> p m h d", m=4, d=DK))

            def proj_chunks(s):
                for n in range(NT):
                    yield lambda n=n: q_chunk(s, n)
                for c in range(NKC[s]):
                    yield lambda c=c: k_chunk(s, c)
                    yield lambda c=c: v_chunk(s, c)

            def attn_tq(s, tq):
                """QK^T + exp + P@V for one (slot, 512-wide Tq chunk),
                plus PV-psum evacuation (frees the pv bufs for the next
                unit) and the den-row gather. Returns (uo_h0, uo_h1, dn)."""
                J = J_list[s]
                pvs = [ps_pv.tile([P, 512], F32, tag="pv", name=f"pv{h}")
                       for h in range(2)]
                for j in range(J):
                    ps = ps_qk.tile([P, 2, 512], F32, tag="qk")
                    nc.tensor.matmul(ps[:, 0, :],
                                     KT[s][0:DK, ts(j, P)],
                                     QT[s][0:DK, ts(tq, 512)],
                                     start=True, stop=True,
                                     tile_position=(0, 0))
                    nc.tensor.matmul(ps[:, 1, :],
                                     KT[s][DK:P, ts(j, P)],
                                     QT[s][DK:P, ts(tq, 512)],
                                     start=True, stop=True,
                                     tile_position=(DK, 0))
                    at = atp.tile([P, 2, 512], dt_x, tag="at")
                    nc.scalar.activation(at[:, :, :], ps[:, :, :], AF.Exp,
                                         bias=mb[s][:, j:j + 1], scale=0.125)
                    for h in range(2):
                        nc.tensor.matmul(pvs[h][0:DK + 1, :],
                                         V[s][:, j, h, :], at[:, h, :],
                                         start=(j == 0), stop=(j == J - 1))
                uos = []
                dn = smp.tile([32, 32], F32, tag="dn")
                for h in range(2):
                    uo = uop.tile([DK + 1, 512], F32, tag="uo")
                    evac(uo[:], pvs[h][0:DK + 1, :], on_act=(h == 0))
                    nc.sync.dma_start(dn[16 * h:16 * h + 16, :],
                                      uo[DK:DK + 1, :])
                    uos.append(uo)
                return uos, dn

            def norm_out(s, tq, unit):
                """normalize + output-project one (slot, Tq chunk)."""
                uos, dn = unit
                rc = smp.tile([32, 32], F32, tag="rc")
                with nc.allow_low_precision(
                        reason="f32r output is bit-identical to f32"):
                    nc.vector.reciprocal(rc[:], dn[:])
                ao = aop.tile([P, 512], dt_x, tag="ao")
                for h in range(2):
                    rst = smp.tile([1, 512], F32R, tag="rst")
                    nc.sync.dma_start(
                        rst[:], rc[16 * h:16 * h + 16, :].bitcast(F32R))
                    ps_b = ps_pv.tile([P, 512], F32, tag="pv", name="ps_b")
                    nc.tensor.matmul(ps_b[0:DK, :], ones_b[0:1, :],
                                     rst[0:1, :], start=True, stop=True)
                    nc.vector.tensor_mul(out=ao[ts(h, DK), :],
                                         in0=ps_b[0:DK, :],
                                         in1=uos[h][0:DK, :])
                ot = otp.tile([P, 4, D], BF16, tag="ot")
                for ml in range(4):
                    for n2 in range(2):
                        ps_o = ps_ms.tile([P, 512], F32, tag="ms")
                        nc.tensor.matmul(ps_o[:], ao[:, ts(ml, P)],
                                         wo[:, ts(n2, 512)],
                                         start=True, stop=True)
                        ev_ct[0] += 1
                        evac(ot[:, ml, ts(n2, 512)], ps_o[:],
                             on_act=(ev_ct[0] % OP_EVAC_ACT_EVERY == 0))
                nc.sync.dma_start(
                    o_d[s][ts(tq, 512), :].rearrange("(m p) d -> p m d", p=P),
                    ot[:])

            # ---- emission schedule --------------------------------------
            # slot 0 projected up front; later slots' projection chunks
            # drain 2 per (slot, tq) unit so PE always has independent
            # filler work while ACT paces the attention stream
            for fn in proj_chunks(0):
                fn()
            pend = {s: list(proj_chunks(s)) for s in range(1, NB)}

            def drain(n):
                while n > 0:
                    s2 = next((x for x in sorted(pend) if pend[x]), None)
                    if s2 is None:
                        return
                    pend[s2].pop(0)()
                    n -= 1

            # software pipeline: unit u's normalization + output
            # projection is emitted 1-2 units later (deeper for small-J
            # slots), so its serial latency chain (DMA->recip->DMA->bcast->
            # mul->MMs) hides under following units' exp streams instead of
            # stalling every engine at each unit boundary
            pnorms = []
            for s in range(NB):
                for tq in range(NT):
                    unit = attn_tq(s, tq)
                    drain(2)
                    if tq == NT - 1 and (s + 1) in pend:
                        # slot s+1 starts next: its chunks must all be out
                        drain(len(pend[s + 1]))
                        del pend[s + 1]
                    pnorms.append((s, tq, unit))
                    skew = 1 if J_list[s] >= 8 else 2
                    while len(pnorms) > skew:
                        norm_out(*pnorms.pop(0))
            for pn in pnorms:
                norm_out(*pn)

    _split_multi_waits(nc)
    return nc


_CACHE = {}


def _get_nc(NB, J_list, dt_x):
    key = (NB, tuple(J_list), str(dt_x))
    if key not in _CACHE:
        _CACHE[key] = build_nc(NB, J_list, dt_x)
    return _CACHE[key]


def _xt(x, dt_np, nchunks=NT):
    """[T, D] -> [nchunks, P, KC, 512] transposed window-major layout."""
    xt = x.T.reshape(KC, P, NT, 512).transpose(2, 1, 0, 3)[:nchunks]
    return np.ascontiguousarray(xt).astype(dt_np)


def kernel(**inputs):
    query = np.asarray(inputs["query"], dtype=np.float32)
    key = np.asarray(inputs["key"], dtype=np.float32)
    value = np.asarray(inputs["value"], dtype=np.float32)
    vl = np.asarray(inputs["valid_length"]).astype(np.int64)
    W_q = np.asarray(inputs["W_q"], dtype=np.float32)
    b_q = np.asarray(inputs["b_q"], dtype=np.float32)
    W_k = np.asarray(inputs["W_k"], dtype=np.float32)
    b_k = np.asarray(inputs["b_k"], dtype=np.float32)
    W_v = np.asarray(inputs["W_v"], dtype=np.float32)
    b_v = np.asarray(inputs["b_v"], dtype=np.float32)
    W_o = np.asarray(inputs["W_o"], dtype=np.float32)
    b_o = np.asarray(inputs["b_o"], dtype=np.float32)

    B = query.shape[0]
    NB = B
    dt_x = BF16
    dt_np = BF16_NP

    # slot s handles batch order[s]; J (Tk tiles) baked per slot.
    # Ascending J: the longest-attention slot runs LAST so its (ACT-paced)
    # exp stream hides the normalization + output projection of every other
    # slot — the kernel ends with almost no PE/DVE-only tail.
    order = np.argsort(vl, kind="stable")
    J_list = []
    for s in range(NB):
        v = int(vl[order[s]])
        J_list.append(TC if v == 0 else max(1, -(-v // P)))
    NKC = [min(NT, -(-j // 4)) for j in J_list]

    nc = _get_nc(NB, J_list, dt_x)

    # host-side shard prep
    xq_np, xk_np, xv_np, mb_np = [], [], [], []
    for s in range(NB):
        b = int(order[s])
        v = int(vl[b])
        q_b = query[b] if v != 0 else np.zeros_like(query[b])
        xq_np.append(_xt(q_b, dt_np))
        xk_np.append(_xt(key[b], dt_np, NKC[s]))
        xv_np.append(_xt(value[b], dt_np, NKC[s]))
        rows = np.arange(P)[:, None] + P * np.arange(TC)[None, :]
        if v == 0:
            m = np.zeros((P, TC), np.float32)
        else:
            m = np.where(rows < v, 0.0, MASK_NEG).astype(np.float32)
        mb_np.append(m)

    in_maps = []
    for c in range(NCORES):
        c0 = c * CPB
        cols = slice(c0, c0 + CPB)
        im = {
            "wq": np.ascontiguousarray(
                W_q.reshape(KC, P, H * DK).transpose(1, 0, 2)[:, :, cols]
            ).astype(dt_np),
            "wk": np.ascontiguousarray(
                W_k.reshape(KC, P, H * DK).transpose(1, 0, 2)[:, :, cols]
            ).astype(dt_np),
            "wv": np.ascontiguousarray(
                W_v.reshape(KC, P, H * DK).transpose(1, 0, 2)[:, :, cols]
            ).astype(dt_np),
            "wo": np.ascontiguousarray(W_o[cols]).astype(dt_np),
            "bq": np.ascontiguousarray(b_q[cols][:, None]).astype(np.float32),
            "bk": np.ascontiguousarray(b_k[cols][:, None]).astype(np.float32),
            "onesb": np.ones((1, DK), np.float32),
        }
        for s in range(NB):
            im[f"xq{s}"] = xq_np[s]
            im[f"xk{s}"] = xk_np[s]
            im[f"xv{s}"] = xv_np[s]
            im[f"mb{s}"] = mb_np[s]
        in_maps.append(im)

    res = run_bass_kernel_spmd(nc, in_maps, list(range(NCORES)))

    # b_v is not applied on device; softmax weights sum to 1 so it adds
    # exactly b_v @ W_o to every output row
    bo_eff = b_o + b_v @ W_o
    out = np.zeros((B, T, D), np.float32)
    for s in range(NB):
        b = int(order[s])
        acc = np.zeros((T, D), np.float32)
        for c in range(NCORES):
            acc += np.asarray(res.results[c][f"o{s}"]).astype(np.float32)
        out[b] = acc + bo_eff[None, :]
    return out
